# revision 16
# baseline (speedup 1.0000x reference)
"""Distributed Trainium2 kernel for nn_ApaBlock (8 NeuronCores, data-parallel).

Architecture (per core, batch shard of 256 rows):
  Z = relu(X @ W1 + b1)                               (TensorE + DVE/ACT)
  ZT_bcast[p, t, b] = Z^T[t, b]  (replicated over p)  (broadcast DMA, once)
  scan over 8 ranks:
    U^T_t = ZiT * ZT_bcast[t]      (DVE bf16 2x mode, chunked)
    G^T  += P[:,t,:]^T @ U^T_t     (128 accumulating 256-wide matmuls)
    batch stats via ACT accum_out -> PE-transpose to row layout
    cross-core AllGather (sync-BN) in [rows, 128] layout (few DMA
    descriptors -> low latency); coeff math on rows; transpose back
    Zi+1^T = a*G^T + c  fused into the PSUM evacuation (ScalarE)
  Y = BN(sum Zi/8) via closed-form global sums (no extra sync);
  out = relu(relu(Y@W3+b3) + relu(X@W2+b2))

The batch-quadratic out[b,k] = sum_pq Zi[b,p] P[p,q,k] Z[b,q] is evaluated
as (Zi x Z outer product) @ P_flat, so TensorE does ONE pass and the
elementwise work runs in SBUF bf16 (DVE 2x) instead of PSUM f32 (1x).

DMA queue discipline: the sync HWDGE engine carries only the big P
streams; every small latency-critical transfer (stat bounces, collective
in/out) goes on the scalar HWDGE engine so it never queues behind a
512KB chunk.
"""

import os
import sys
import types

if "/opt/trn_rl_repo" not in sys.path:
    sys.path.insert(0, "/opt/trn_rl_repo")

import numpy as np
import ml_dtypes

N_CORES = 8
B, IN, H, OUT, RANK = 2048, 256, 128, 128, 8
BS = B // N_CORES  # 256 rows per core
NBT = BS // 128  # 2 b-tiles per core
EPS = 1e-5
QK = H * H  # 16384
NCH = 8  # P DMA chunks per rank
TCH = H // NCH  # t's (q-planes) per P chunk (16)
UCH = [4, 18, 18, 18, 18, 18, 18, 8, 8]  # U-build chunk sizes (sum 128)

_cache = {}


def _ensure_axon_hooks_shim():
    """bass_utils imports antenv.axon_hooks when BASS_TRACE is set; the agent
    image lacks it. Provide a null shim so tracing degrades gracefully."""
    try:
        import antenv.axon_hooks  # noqa: F401
        return
    except ImportError:
        pass
    try:
        import antenv  # noqa: F401
    except ImportError:
        return
    mod = types.ModuleType("antenv.axon_hooks")
    _state = {"hook": None}
    mod.set_axon_ntff_profile_hook = lambda h: _state.__setitem__("hook", h)
    mod.get_axon_ntff_profile_hook = lambda: _state["hook"]
    sys.modules["antenv.axon_hooks"] = mod


def _build():
    from concourse import bacc, mybir, tile

    f32 = mybir.dt.float32
    bf16 = mybir.dt.bfloat16
    FT = mybir.ActivationFunctionType
    AL = mybir.AluOpType

    nc = bacc.Bacc("TRN2", target_bir_lowering=False, debug=False,
                   num_devices=N_CORES)

    XTd = nc.declare_dram_parameter("XT", [2, 128, BS], bf16, isOutput=False)
    Pd = nc.declare_dram_parameter("P", [RANK, H, QK], bf16, isOutput=False)
    W1d = nc.declare_dram_parameter("W1", [2, 128, H], bf16, isOutput=False)
    W2d = nc.declare_dram_parameter("W2", [2, 128, OUT], bf16, isOutput=False)
    W3d = nc.declare_dram_parameter("W3", [H, OUT], bf16, isOutput=False)
    B1d = nc.declare_dram_parameter("b1b", [128, H], f32, isOutput=False)
    B2d = nc.declare_dram_parameter("b2b", [128, OUT], f32, isOutput=False)
    B3d = nc.declare_dram_parameter("b3b", [128, OUT], f32, isOutput=False)
    BNRd = nc.declare_dram_parameter("bnr", [1, 4 * H], f32, isOutput=False)
    IDd = nc.declare_dram_parameter("ident", [128, 128], bf16, isOutput=False)
    IDFd = nc.declare_dram_parameter("identf", [128, 128], f32,
                                     isOutput=False)
    OUTd = nc.declare_dram_parameter("out", [BS, OUT], f32, isOutput=True)

    rg = [list(range(N_CORES))]
    CW = TCH * 128  # P columns per chunk (2048)

    with tile.TileContext(nc) as tc:
        with (
            tc.tile_pool(name="const", bufs=1) as cpool,
            tc.tile_pool(name="ppool", bufs=1) as ppool,
            tc.tile_pool(name="upool", bufs=1) as upool,
            tc.tile_pool(name="zit", bufs=2) as zitpool,
            tc.tile_pool(name="small", bufs=2) as spool,
            tc.tile_pool(name="psg", bufs=2, space="PSUM") as psg,
            tc.tile_pool(name="psmm", bufs=2, space="PSUM") as psmm,
            tc.tile_pool(name="pstr", bufs=1, space="PSUM") as pstr,
            tc.tile_pool(name="dram", bufs=4, space="DRAM") as dpool,
        ):
            # ---- constants first (small; unblock Z), then P-rank0 ----
            xt = cpool.tile([128, 2 * BS], bf16, tag="xt")
            for c in range(2):
                nc.sync.dma_start(xt[:, c * BS:(c + 1) * BS], XTd[c])
            w1 = cpool.tile([128, 2 * H], bf16, tag="w1")
            w2 = cpool.tile([128, 2 * OUT], bf16, tag="w2")
            for c in range(2):
                nc.scalar.dma_start(w1[:, c * H:(c + 1) * H], W1d[c])
                nc.scalar.dma_start(w2[:, c * OUT:(c + 1) * OUT], W2d[c])
            w3 = cpool.tile([H, OUT], bf16, tag="w3")
            nc.scalar.dma_start(w3[:], W3d[:])
            b1b = cpool.tile([128, H], f32, tag="b1b")
            b2b = cpool.tile([128, OUT], f32, tag="b2b")
            b3b = cpool.tile([128, OUT], f32, tag="b3b")
            nc.scalar.dma_start(b1b[:], B1d[:])
            nc.scalar.dma_start(b2b[:], B2d[:])
            nc.scalar.dma_start(b3b[:], B3d[:])
            bnf = cpool.tile([1, 4 * H], f32, tag="bnf")
            nc.scalar.dma_start(bnf[:], BNRd[:])
            ones11 = cpool.tile([1, 1], f32, tag="ones11")
            nc.vector.memset(ones11[:], 1.0)
            ident = cpool.tile([128, 128], bf16, tag="ident")
            nc.scalar.dma_start(ident[:], IDd[:])
            identf = cpool.tile([128, 128], f32, tag="identf")
            nc.scalar.dma_start(identf[:], IDFd[:])
            epsr = cpool.tile([1, 1], f32, tag="epsr")
            nc.vector.memset(epsr[:], EPS)

            yt = cpool.tile([H, BS], f32, tag="yt")  # Y^T accumulator
            nc.vector.memset(yt[:], 0.0)

            # rank-0 P prefetch: sync HWDGE is dedicated to P traffic.
            p_ch = [ppool.tile([128, CW], bf16, tag=f"p{c}", name=f"p{c}")
                    for c in range(NCH)]
            for c in range(NCH):
                nc.sync.dma_start(p_ch[c][:], Pd[0][:, c * CW:(c + 1) * CW])

            # Early dummy collective: absorbs cross-core launch skew and the
            # ncfw first-call overhead while the engines do setup + rank-0.
            dsrc = dpool.tile([1, 2 * H], f32, tag="ccsrcd")
            ddst = dpool.tile([N_CORES, 2 * H], f32, tag="ccdstd")
            nc.scalar.dma_start(dsrc[:], bnf[0:1, 0:2 * H])
            nc.gpsimd.collective_compute(
                "AllGather", AL.bypass, replica_groups=rg,
                ins=[dsrc.opt()], outs=[ddst.opt()],
            )

            # ---------------- Z = relu(X@W1 + b1) ----------------
            zb = cpool.tile([128, 2 * H], bf16, tag="zb")  # Z, b-partition
            for bt in range(NBT):
                ps = psmm.tile([128, H], f32, tag="mm")
                for c in range(2):
                    nc.tensor.matmul(
                        ps[:],
                        lhsT=xt[:, c * BS + bt * 128: c * BS + (bt + 1) * 128],
                        rhs=w1[:, c * H:(c + 1) * H],
                        start=(c == 0), stop=(c == 1),
                    )
                t0 = spool.tile([128, H], f32, tag="ztmp")
                nc.vector.tensor_tensor(t0[:], ps[:], b1b[:], AL.add)
                nc.scalar.activation(zb[:, bt * H:(bt + 1) * H], t0[:],
                                     FT.Relu)

            # Z^T (q-part, b): initial Zi^T, and the source for ZT_bcast
            zt = cpool.tile([H, BS], bf16, tag="zt")
            for bt in range(NBT):
                pst = pstr.tile([128, 128], bf16, tag="tr")
                nc.tensor.transpose(pst[:],
                                    zb[:, bt * H:(bt + 1) * H], ident[:])
                nc.scalar.activation(zt[:, bt * 128:(bt + 1) * 128],
                                     pst[:], FT.Copy)

            # ZT_bcast[p, (t, b)] = ZT[t, b]: bounce ZT to DRAM, then
            # broadcast-read it back into all 128 partitions (chunked, on
            # both engines, so rank-0 U-builds can start on chunk 0 early).
            dzt = dpool.tile([H, BS], bf16, tag="dzt")
            nc.scalar.dma_start(dzt[:], zt[:])
            ztb = cpool.tile([128, H * BS], bf16, tag="ztb")
            ztb3 = ztb[:].rearrange("p (t b) -> p t b", b=BS)
            for c in range(NCH):
                src = dzt[c * TCH:(c + 1) * TCH, :].rearrange(
                    "(o t) b -> o t b", o=1).broadcast_to((128, TCH, BS))
                eng = nc.scalar if c % 2 == 0 else nc.sync
                eng.dma_start(ztb3[:, c * TCH:(c + 1) * TCH, :], src)

            # relu(X@W2+b2): sync-independent, fills early TensorE idle
            r2rs = []
            for bt in range(NBT):
                psB = psmm.tile([128, OUT], f32, tag="mm")
                for c in range(2):
                    nc.tensor.matmul(
                        psB[:],
                        lhsT=xt[:, c * BS + bt * 128: c * BS + (bt + 1) * 128],
                        rhs=w2[:, c * OUT:(c + 1) * OUT],
                        start=(c == 0), stop=(c == 1),
                    )
                r2 = spool.tile([128, OUT], f32, tag="r2")
                nc.vector.tensor_tensor(r2[:], psB[:], b2b[:], AL.add)
                r2r = spool.tile([128, OUT], f32, tag=f"r2r{bt}")
                nc.scalar.activation(r2r[:], r2[:], FT.Relu)
                r2rs.append(r2r)

            # ---------------- scan over ranks ----------------
            zit = zt
            gpsum = None
            arow = crow = stg = None
            for r in range(RANK):
                if r > 0:
                    p_ch = [ppool.tile([128, CW], bf16, tag=f"p{c}",
                                       name=f"p{c}")
                            for c in range(NCH)]
                    for c in range(NCH):
                        nc.sync.dma_start(p_ch[c][:],
                                          Pd[r][:, c * CW:(c + 1) * CW])

                gpsum = psg.tile([128, BS], f32, tag="g")
                t0c = 0
                for j, csz in enumerate(UCH):
                    # U^T chunk: ut[p, t, b] = ZiT[p, b] * ZT[t, b]
                    zin = zit[:].rearrange("p (o b) -> p o b", o=1
                                           ).broadcast_to((128, csz, BS))
                    ut = upool.tile([128, 18 * BS], bf16, tag=f"u{j % 4}",
                                    name=f"u{j % 4}")
                    ut3 = ut[:, 0:csz * BS].rearrange("p (t b) -> p t b",
                                                      b=BS)
                    nc.vector.tensor_tensor(
                        ut3, zin, ztb3[:, t0c:t0c + csz, :], AL.mult)
                    for i in range(csz):
                        t = t0c + i
                        pc, pi = t // TCH, t % TCH
                        nc.tensor.matmul(
                            gpsum[:],
                            lhsT=p_ch[pc][:, pi * 128:(pi + 1) * 128],
                            rhs=ut3[:, i, :],
                            start=(t == 0), stop=(t == H - 1),
                        )
                    t0c += csz

                # batch stats straight from PSUM via ACT accum_out
                last = (r == RANK - 1)
                stw = 8 if last else 2
                stl = spool.tile([H, stw], f32, tag=f"stl{stw}")
                if last:
                    nc.vector.memset(stl[:], 0.0)
                scr = spool.tile([H, BS], bf16, tag="scr")
                if last:
                    gt = spool.tile([H, BS], bf16, tag="gt")
                    nc.scalar.activation(gt[:], gpsum[:], FT.Copy,
                                         accum_out=stl[:, 0:1])
                else:
                    nc.scalar.activation(scr[:], gpsum[:], FT.Copy,
                                         accum_out=stl[:, 0:1])
                nc.scalar.activation(scr[:], gpsum[:], FT.Square,
                                     accum_out=stl[:, 1:2])
                if last:
                    # piggyback Y-BN inputs on the final sync: with
                    # R = sum_{i<8} Zi (= yt now) and Zi8 = a*G + c,
                    # SumY and SumY^2 expand in closed form from
                    # [S1G, S2G, S1R, S2R, Sum(R*G)] -- no 9th sync.
                    nc.scalar.activation(scr[:], yt[:], FT.Copy,
                                         accum_out=stl[:, 2:3])
                    nc.scalar.activation(scr[:], yt[:], FT.Square,
                                         accum_out=stl[:, 3:4])
                    scry2 = spool.tile([H, BS], bf16, tag="scry2")
                    nc.vector.tensor_tensor(scry2[:], yt[:], gt[:], AL.mult)
                    nc.scalar.activation(scr[:], scry2[:], FT.Copy,
                                         accum_out=stl[:, 4:5])

                # flatten stats into a single partition-0 row [1, stw*128]
                # (the cross-core bounce then needs only 1 DMA descriptor
                # instead of 16 partition-group descriptors = ~6us saved)
                strow = spool.tile([1, 8 * 128], f32, tag="strow",
                                   bufs=1)
                for g in range((stw + 3) // 4):
                    ncol = min(4, stw - 4 * g)
                    pstt = pstr.tile([1, 512], f32, tag="trs", name="pstt")
                    for s4 in range(ncol):
                        s = 4 * g + s4
                        nc.tensor.matmul(pstt[0:1, s4 * 128:(s4 + 1) * 128],
                                         lhsT=stl[:, s:s + 1], rhs=identf[:],
                                         start=True, stop=True)
                    nc.scalar.activation(
                        strow[0:1, g * 512:g * 512 + ncol * 128],
                        pstt[0:1, 0:ncol * 128], FT.Copy)

                # ---- cross-core AllGather of row stats + coeffs ----
                arow, crow, stg, acp = _bn_sync(nc, tc, dpool, spool, pstr,
                                                strow, stw, bnf, ones11,
                                                epsr=epsr)

                # BN apply fused into the PSUM evacuation:
                # Zi+1^T = a*G^T + c  (per-partition affine on ScalarE)
                zit_next = zitpool.tile([H, BS], bf16, tag="zit")
                nc.scalar.activation(zit_next[:], gpsum[:], FT.Identity,
                                     bias=acp[:, 1:2], scale=acp[:, 0:1])
                nc.vector.tensor_tensor(yt[:], yt[:], zit_next[:], AL.add)
                zit = zit_next

            # ------- Y BN from closed-form global sums (no extra sync) ----
            # stg (partition-0 row, stride 128): [S1G, S2G, S1R, S2R, SX];
            # arow/crow = rank-7 BN coeff row-slices.
            # SumY  = (S1R + a*S1G + B*c) / 8
            # SumY2 = (S2R + 2*(a*SX + c*S1R)
            #          + a^2*S2G + 2*a*c*S1G + B*c^2) / 64
            def _sl(tile, i):
                return tile[0:1, i * H:(i + 1) * H]

            S1G, S2G = _sl(stg, 0), _sl(stg, 1)
            S1R, S2R = _sl(stg, 2), _sl(stg, 3)
            SX = _sl(stg, 4)
            w = spool.tile([1, 10 * H], f32, tag="ywork", bufs=1)
            w0, w1_, w2_, w3_, w4 = (_sl(w, i) for i in range(5))
            w5, w6, w7, w8, w9 = (_sl(w, i) for i in range(5, 10))
            nc.vector.tensor_tensor(w0, arow, S1G, AL.mult)   # a*S1G
            nc.vector.tensor_scalar(w1_, crow, float(B), None, AL.mult)
            nc.vector.tensor_tensor(w1_, w1_, w0, AL.add)
            nc.vector.tensor_tensor(w2_, w1_, S1R, AL.add)    # 8*SumY
            nc.vector.tensor_tensor(w3_, arow, SX, AL.mult)
            nc.vector.tensor_tensor(w4, crow, S1R, AL.mult)
            nc.vector.tensor_tensor(w3_, w3_, w4, AL.add)     # SRZ
            nc.vector.tensor_tensor(w5, arow, arow, AL.mult)  # a^2
            nc.vector.tensor_tensor(w5, w5, S2G, AL.mult)
            nc.vector.tensor_tensor(w6, arow, crow, AL.mult)  # a*c
            nc.vector.tensor_tensor(w6, w6, S1G, AL.mult)
            nc.vector.tensor_tensor(w7, crow, crow, AL.mult)  # c^2
            nc.vector.tensor_scalar(w7, w7, float(B), None, AL.mult)
            # S2Z = a^2*S2G + 2*a*c*S1G + B*c^2
            nc.vector.tensor_scalar(w6, w6, 2.0, None, AL.mult)
            nc.vector.tensor_tensor(w5, w5, w6, AL.add)
            nc.vector.tensor_tensor(w5, w5, w7, AL.add)
            nc.vector.tensor_scalar(w3_, w3_, 2.0, None, AL.mult)
            nc.vector.tensor_tensor(w8, S2R, w3_, AL.add)
            nc.vector.tensor_tensor(w8, w8, w5, AL.add)       # 64*SumY2
            nc.vector.tensor_scalar(w2_, w2_, 1.0 / (8.0 * B), None,
                                    AL.mult)                  # mY
            nc.vector.tensor_scalar(w8, w8, 1.0 / (64.0 * B), None,
                                    AL.mult)                  # E[Y^2]
            nc.vector.tensor_tensor(w9, w2_, w2_, AL.mult)
            nc.vector.tensor_scalar(w9, w9, -1.0, None, AL.mult)
            nc.vector.tensor_tensor(w9, w9, w8, AL.add)       # var
            sdy = spool.tile([1, 4 * H], f32, tag="ycoef", bufs=1)
            sd0, ay, cy, ay8 = (_sl(sdy, i) for i in range(4))
            nc.scalar.activation(sd0, w9, FT.Sqrt, bias=epsr[:])
            nc.vector.reciprocal(ay, sd0)
            nc.vector.tensor_tensor(ay, ay, bnf[0:1, 2 * H:3 * H],
                                    AL.mult)                  # ay
            nc.vector.tensor_tensor(cy, w2_, ay, AL.mult)
            nc.vector.tensor_tensor(cy, bnf[0:1, 3 * H:4 * H], cy,
                                    AL.subtract)              # cy
            nc.vector.tensor_scalar(ay8, ay, 0.125, None, AL.mult)
            # [ay/8, cy] rows -> per-partition [128, 2] via c=1 matmuls
            psty = pstr.tile([128, 2], f32, tag="trb")
            nc.tensor.matmul(psty[:, 0:1], lhsT=ay8, rhs=ones11[:],
                             start=True, stop=True)
            nc.tensor.matmul(psty[:, 1:2], lhsT=cy, rhs=ones11[:],
                             start=True, stop=True)
            acy = spool.tile([128, 2], f32, tag="acy")
            nc.scalar.activation(acy[:], psty[:], FT.Copy)
            ybn = spool.tile([H, BS], bf16, tag="ybn")
            nc.vector.tensor_scalar(ybn[:], yt[:], acy[:, 0:1], acy[:, 1:2],
                                    AL.mult, AL.add)

            # ---------------- final: relu(relu(Y@W3+b3)+relu(X@W2+b2)) ----
            for bt in range(NBT):
                psA = psmm.tile([128, OUT], f32, tag="mm")
                nc.tensor.matmul(psA[:],
                                 lhsT=ybn[:, bt * 128:(bt + 1) * 128],
                                 rhs=w3[:], start=True, stop=True)
                r1 = spool.tile([128, OUT], f32, tag="r1")
                nc.vector.tensor_tensor(r1[:], psA[:], b3b[:], AL.add)
                r1r = spool.tile([128, OUT], f32, tag="r1r")
                nc.scalar.activation(r1r[:], r1[:], FT.Relu)

                s = spool.tile([128, OUT], f32, tag="s")
                nc.vector.tensor_tensor(s[:], r1r[:], r2rs[bt][:], AL.add)
                of = spool.tile([128, OUT], f32, tag="of")
                nc.scalar.activation(of[:], s[:], FT.Relu)
                nc.scalar.dma_start(OUTd[bt * 128:(bt + 1) * 128, :],
                                    of[:])

    nc.compile()
    return nc


def _bn_sync(nc, tc, dpool, spool, pstr, strow, stw, bnf, ones11,
             epsr=None):
    """AllGather per-core [1, stw*128] row stats (single-descriptor DMAs),
    tree-reduce across the 8 cores along the free dim, compute affine
    coeff rows a, c s.t. BN(x) = a*x + c, and push (a, c) back out to
    per-partition [128, 2] for the fused PSUM evacuation.

    Returns (a_row, c_row, global-sum row tile, acp[128, 2])."""
    from concourse import mybir

    f32 = mybir.dt.float32
    FT = mybir.ActivationFunctionType
    AL = mybir.AluOpType

    W = stw * 128
    src = dpool.tile([1, W], f32, tag=f"ccsrc{stw}")
    dst = dpool.tile([N_CORES, W], f32, tag=f"ccdst{stw}")
    nc.scalar.dma_start(src[:], strow[0:1, 0:W])
    nc.gpsimd.collective_compute(
        "AllGather", AL.bypass, replica_groups=[list(range(N_CORES))],
        ins=[src.opt()], outs=[dst.opt()],
    )
    gath = spool.tile([1, N_CORES * 8 * 128], f32, tag="gath", bufs=1)
    nc.scalar.dma_start(
        gath[0:1, 0:N_CORES * W],
        dst[:].rearrange("(o c) w -> o (c w)", o=1))
    # tree-reduce over cores along the free dim, in place
    nc.vector.tensor_tensor(gath[0:1, 0:4 * W], gath[0:1, 0:4 * W],
                            gath[0:1, 4 * W:8 * W], AL.add)
    nc.vector.tensor_tensor(gath[0:1, 0:2 * W], gath[0:1, 0:2 * W],
                            gath[0:1, 2 * W:4 * W], AL.add)
    nc.vector.tensor_tensor(gath[0:1, 0:W], gath[0:1, 0:W],
                            gath[0:1, W:2 * W], AL.add)
    st = gath[0:1, 0:W]

    cf = spool.tile([1, 2 * 128], f32, tag="cf", bufs=1)
    m = cf[0:1, 0:128]
    ex2 = cf[0:1, 128:256]
    work = spool.tile([1, 3 * 128], f32, tag="cfw", bufs=1)
    v = work[0:1, 0:128]
    sd = work[0:1, 128:256]
    rinv = work[0:1, 256:384]
    nc.vector.tensor_scalar(cf[0:1, :], st[0:1, 0:256], 1.0 / B, None,
                            AL.mult)

    nc.vector.tensor_tensor(v, m, m, AL.mult)
    nc.vector.tensor_scalar(v, v, -1.0, None, AL.mult)
    nc.vector.tensor_tensor(v, v, ex2, AL.add)
    nc.scalar.activation(sd, v, FT.Sqrt, bias=epsr[:])
    nc.vector.reciprocal(rinv, sd)
    acr = spool.tile([1, 2 * 128], f32, tag="acr", bufs=1)
    a = acr[0:1, 0:128]
    c = acr[0:1, 128:256]
    nc.vector.tensor_tensor(a, rinv, bnf[0:1, 0:128], AL.mult)
    nc.vector.tensor_tensor(c, m, a, AL.mult)
    nc.vector.tensor_tensor(c, bnf[0:1, 128:256], c, AL.subtract)
    # (a, c) row -> per-partition [128, 2] via two contraction-1 matmuls
    pst2 = pstr.tile([128, 2], f32, tag="trb")
    nc.tensor.matmul(pst2[:, 0:1], lhsT=a, rhs=ones11[:],
                     start=True, stop=True)
    nc.tensor.matmul(pst2[:, 1:2], lhsT=c, rhs=ones11[:],
                     start=True, stop=True)
    acp = spool.tile([128, 2], f32, tag="acp")
    nc.scalar.activation(acp[:], pst2[:], FT.Copy)
    return a, c, st, acp


def _prep_inputs(X, W1, b1, W2, b2, W3, b3, P, gz, bz, gy, by):
    bf = ml_dtypes.bfloat16
    per_core = []
    P_b = np.ascontiguousarray(P.reshape(RANK, H, QK)).astype(bf)
    W1_b = np.ascontiguousarray(W1.reshape(2, 128, H)).astype(bf)
    W2_b = np.ascontiguousarray(W2.reshape(2, 128, OUT)).astype(bf)
    W3_b = np.ascontiguousarray(W3).astype(bf)
    b1b = np.broadcast_to(b1, (128, H)).astype(np.float32).copy()
    b2b = np.broadcast_to(b2, (128, OUT)).astype(np.float32).copy()
    b3b = np.broadcast_to(b3, (128, OUT)).astype(np.float32).copy()
    bnr = np.concatenate([gz, bz, gy, by]).reshape(1, 4 * H
                                                   ).astype(np.float32)
    ident = np.eye(128, dtype=np.float32).astype(bf)
    identf = np.eye(128, dtype=np.float32)
    for s in range(N_CORES):
        Xs = X[s * BS:(s + 1) * BS]
        XT = np.ascontiguousarray(Xs.T.reshape(2, 128, BS)).astype(bf)
        per_core.append({
            "XT": XT, "P": P_b, "W1": W1_b, "W2": W2_b, "W3": W3_b,
            "b1b": b1b, "b2b": b2b, "b3b": b3b, "bnr": bnr, "ident": ident,
            "identf": identf,
        })
    return per_core


def kernel(**inputs):
    _ensure_axon_hooks_shim()
    from concourse.bass_utils import run_bass_kernel_spmd

    if "nc" not in _cache:
        _cache["nc"] = _build()
    nc = _cache["nc"]

    in_maps = _prep_inputs(**{k: np.asarray(v) for k, v in inputs.items()})
    res = run_bass_kernel_spmd(nc, in_maps, core_ids=list(range(N_CORES)))
    out = np.concatenate([m["out"] for m in res.results], axis=0)
    return out.astype(np.float32)


if __name__ == "__main__":
    import reference as R

    inputs = {k: np.asarray(v) for k, v in R.setup_inputs().items()}
    got = kernel(**inputs)
    exp = np.asarray(R.reference(**R.setup_inputs()))
    rel = np.linalg.norm(got - exp) / np.linalg.norm(exp)
    print("rel l2:", rel)


# revision 17
# speedup vs baseline: 1.0286x; 1.0286x over previous
"""Distributed Trainium2 kernel for nn_ApaBlock (8 NeuronCores, data-parallel).

Architecture (per core, batch shard of 256 rows):
  Z = relu(X @ W1 + b1)                               (TensorE + DVE/ACT)
  ZT_bcast[p, t, b] = Z^T[t, b]  (replicated over p)  (broadcast DMA, once)
  scan over 8 ranks:
    U^T_t = ZiT * ZT_bcast[t]      (DVE bf16 2x mode, chunked)
    G^T  += P[:,t,:]^T @ U^T_t     (128 accumulating 256-wide matmuls)
    batch stats via ACT accum_out -> PE-transpose to row layout
    cross-core AllGather (sync-BN) in [rows, 128] layout (few DMA
    descriptors -> low latency); coeff math on rows; transpose back
    Zi+1^T = a*G^T + c  fused into the PSUM evacuation (ScalarE)
  Y = BN(sum Zi/8) via closed-form global sums (no extra sync);
  out = relu(relu(Y@W3+b3) + relu(X@W2+b2))

The batch-quadratic out[b,k] = sum_pq Zi[b,p] P[p,q,k] Z[b,q] is evaluated
as (Zi x Z outer product) @ P_flat, so TensorE does ONE pass and the
elementwise work runs in SBUF bf16 (DVE 2x) instead of PSUM f32 (1x).

DMA queue discipline: the sync HWDGE engine carries only the big P
streams; every small latency-critical transfer (stat bounces, collective
in/out) goes on the scalar HWDGE engine so it never queues behind a
512KB chunk.
"""

import os
import sys
import types

if "/opt/trn_rl_repo" not in sys.path:
    sys.path.insert(0, "/opt/trn_rl_repo")

import numpy as np
import ml_dtypes

N_CORES = 8
B, IN, H, OUT, RANK = 2048, 256, 128, 128, 8
BS = B // N_CORES  # 256 rows per core
NBT = BS // 128  # 2 b-tiles per core
EPS = 1e-5
QK = H * H  # 16384
NCH = 8  # P DMA chunks per rank
TCH = H // NCH  # t's (q-planes) per P chunk (16)
UCH = [4, 18, 18, 18, 18, 18, 18, 8, 8]  # U-build chunk sizes (sum 128)

_cache = {}


def _ensure_axon_hooks_shim():
    """bass_utils imports antenv.axon_hooks when BASS_TRACE is set; the agent
    image lacks it. Provide a null shim so tracing degrades gracefully."""
    try:
        import antenv.axon_hooks  # noqa: F401
        return
    except ImportError:
        pass
    try:
        import antenv  # noqa: F401
    except ImportError:
        return
    mod = types.ModuleType("antenv.axon_hooks")
    _state = {"hook": None}
    mod.set_axon_ntff_profile_hook = lambda h: _state.__setitem__("hook", h)
    mod.get_axon_ntff_profile_hook = lambda: _state["hook"]
    sys.modules["antenv.axon_hooks"] = mod


def _build():
    from concourse import bacc, mybir, tile

    f32 = mybir.dt.float32
    bf16 = mybir.dt.bfloat16
    FT = mybir.ActivationFunctionType
    AL = mybir.AluOpType

    nc = bacc.Bacc("TRN2", target_bir_lowering=False, debug=False,
                   num_devices=N_CORES)

    XTd = nc.declare_dram_parameter("XT", [2, 128, BS], bf16, isOutput=False)
    Pd = nc.declare_dram_parameter("P", [RANK, H, QK], bf16, isOutput=False)
    W1d = nc.declare_dram_parameter("W1", [2, 128, H], bf16, isOutput=False)
    W2d = nc.declare_dram_parameter("W2", [2, 128, OUT], bf16, isOutput=False)
    W3d = nc.declare_dram_parameter("W3", [H, OUT], bf16, isOutput=False)
    B1d = nc.declare_dram_parameter("b1b", [128, H], f32, isOutput=False)
    B2d = nc.declare_dram_parameter("b2b", [128, OUT], f32, isOutput=False)
    B3d = nc.declare_dram_parameter("b3b", [128, OUT], f32, isOutput=False)
    BNRd = nc.declare_dram_parameter("bnr", [1, 4 * H], f32, isOutput=False)
    BNCd = nc.declare_dram_parameter("bnc", [H, 4], f32, isOutput=False)
    IDd = nc.declare_dram_parameter("ident", [128, 128], bf16, isOutput=False)
    IDFd = nc.declare_dram_parameter("identf", [128, 128], f32,
                                     isOutput=False)
    OUTd = nc.declare_dram_parameter("out", [BS, OUT], f32, isOutput=True)

    rg = [list(range(N_CORES))]
    CW = TCH * 128  # P columns per chunk (2048)

    with tile.TileContext(nc) as tc:
        with (
            tc.tile_pool(name="const", bufs=1) as cpool,
            tc.tile_pool(name="ppool", bufs=1) as ppool,
            tc.tile_pool(name="upool", bufs=1) as upool,
            tc.tile_pool(name="zit", bufs=2) as zitpool,
            tc.tile_pool(name="small", bufs=2) as spool,
            tc.tile_pool(name="psg", bufs=2, space="PSUM") as psg,
            tc.tile_pool(name="psmm", bufs=2, space="PSUM") as psmm,
            tc.tile_pool(name="pstr", bufs=1, space="PSUM") as pstr,
            tc.tile_pool(name="dram", bufs=4, space="DRAM") as dpool,
        ):
            # ---- constants first (small; unblock Z), then P-rank0 ----
            xt = cpool.tile([128, 2 * BS], bf16, tag="xt")
            for c in range(2):
                nc.sync.dma_start(xt[:, c * BS:(c + 1) * BS], XTd[c])
            w1 = cpool.tile([128, 2 * H], bf16, tag="w1")
            w2 = cpool.tile([128, 2 * OUT], bf16, tag="w2")
            for c in range(2):
                nc.scalar.dma_start(w1[:, c * H:(c + 1) * H], W1d[c])
                nc.scalar.dma_start(w2[:, c * OUT:(c + 1) * OUT], W2d[c])
            w3 = cpool.tile([H, OUT], bf16, tag="w3")
            nc.scalar.dma_start(w3[:], W3d[:])
            b1b = cpool.tile([128, H], f32, tag="b1b")
            b2b = cpool.tile([128, OUT], f32, tag="b2b")
            b3b = cpool.tile([128, OUT], f32, tag="b3b")
            nc.scalar.dma_start(b1b[:], B1d[:])
            nc.scalar.dma_start(b2b[:], B2d[:])
            nc.scalar.dma_start(b3b[:], B3d[:])
            bnf = cpool.tile([1, 4 * H], f32, tag="bnf")
            nc.scalar.dma_start(bnf[:], BNRd[:])
            ones11 = cpool.tile([1, 1], f32, tag="ones11")
            nc.vector.memset(ones11[:], 1.0)
            bnc = cpool.tile([H, 4], f32, tag="bnc")
            nc.scalar.dma_start(bnc[:], BNCd[:])
            epsc = cpool.tile([H, 1], f32, tag="epsc")
            nc.vector.memset(epsc[:], EPS)
            ident = cpool.tile([128, 128], bf16, tag="ident")
            nc.scalar.dma_start(ident[:], IDd[:])
            identf = cpool.tile([128, 128], f32, tag="identf")
            nc.scalar.dma_start(identf[:], IDFd[:])
            epsr = cpool.tile([1, 1], f32, tag="epsr")
            nc.vector.memset(epsr[:], EPS)

            yt = cpool.tile([H, BS], f32, tag="yt")  # Y^T accumulator
            nc.vector.memset(yt[:], 0.0)

            # rank-0 P prefetch: sync HWDGE is dedicated to P traffic.
            p_ch = [ppool.tile([128, CW], bf16, tag=f"p{c}", name=f"p{c}")
                    for c in range(NCH)]
            for c in range(NCH):
                nc.sync.dma_start(p_ch[c][:], Pd[0][:, c * CW:(c + 1) * CW])

            # Early dummy collective: absorbs cross-core launch skew and the
            # ncfw first-call overhead while the engines do setup + rank-0.
            dsrc = dpool.tile([1, 2 * H], f32, tag="ccsrcd")
            ddst = dpool.tile([N_CORES, 2 * H], f32, tag="ccdstd")
            nc.scalar.dma_start(dsrc[:], bnf[0:1, 0:2 * H])
            nc.gpsimd.collective_compute(
                "AllGather", AL.bypass, replica_groups=rg,
                ins=[dsrc.opt()], outs=[ddst.opt()],
            )

            # ---------------- Z = relu(X@W1 + b1) ----------------
            zb = cpool.tile([128, 2 * H], bf16, tag="zb")  # Z, b-partition
            for bt in range(NBT):
                ps = psmm.tile([128, H], f32, tag="mm")
                for c in range(2):
                    nc.tensor.matmul(
                        ps[:],
                        lhsT=xt[:, c * BS + bt * 128: c * BS + (bt + 1) * 128],
                        rhs=w1[:, c * H:(c + 1) * H],
                        start=(c == 0), stop=(c == 1),
                    )
                t0 = spool.tile([128, H], f32, tag="ztmp")
                nc.vector.tensor_tensor(t0[:], ps[:], b1b[:], AL.add)
                nc.scalar.activation(zb[:, bt * H:(bt + 1) * H], t0[:],
                                     FT.Relu)

            # Z^T (q-part, b): initial Zi^T, and the source for ZT_bcast
            zt = cpool.tile([H, BS], bf16, tag="zt")
            for bt in range(NBT):
                pst = pstr.tile([128, 128], bf16, tag="tr")
                nc.tensor.transpose(pst[:],
                                    zb[:, bt * H:(bt + 1) * H], ident[:])
                nc.scalar.activation(zt[:, bt * 128:(bt + 1) * 128],
                                     pst[:], FT.Copy)

            # ZT_bcast[p, (t, b)] = ZT[t, b]: bounce ZT to DRAM, then
            # broadcast-read it back into all 128 partitions (chunked, on
            # both engines, so rank-0 U-builds can start on chunk 0 early).
            dzt = dpool.tile([H, BS], bf16, tag="dzt")
            nc.scalar.dma_start(dzt[:], zt[:])
            ztb = cpool.tile([128, H * BS], bf16, tag="ztb")
            ztb3 = ztb[:].rearrange("p (t b) -> p t b", b=BS)
            for c in range(NCH):
                src = dzt[c * TCH:(c + 1) * TCH, :].rearrange(
                    "(o t) b -> o t b", o=1).broadcast_to((128, TCH, BS))
                eng = nc.scalar if c % 2 == 0 else nc.sync
                eng.dma_start(ztb3[:, c * TCH:(c + 1) * TCH, :], src)

            # relu(X@W2+b2): sync-independent, fills early TensorE idle
            r2rs = []
            for bt in range(NBT):
                psB = psmm.tile([128, OUT], f32, tag="mm")
                for c in range(2):
                    nc.tensor.matmul(
                        psB[:],
                        lhsT=xt[:, c * BS + bt * 128: c * BS + (bt + 1) * 128],
                        rhs=w2[:, c * OUT:(c + 1) * OUT],
                        start=(c == 0), stop=(c == 1),
                    )
                r2 = spool.tile([128, OUT], f32, tag="r2")
                nc.vector.tensor_tensor(r2[:], psB[:], b2b[:], AL.add)
                r2r = spool.tile([128, OUT], f32, tag=f"r2r{bt}")
                nc.scalar.activation(r2r[:], r2[:], FT.Relu)
                r2rs.append(r2r)

            # ---------------- scan over ranks ----------------
            zit = zt
            gpsum = None
            arow = crow = stg = None
            for r in range(RANK):
                if r > 0:
                    p_ch = [ppool.tile([128, CW], bf16, tag=f"p{c}",
                                       name=f"p{c}")
                            for c in range(NCH)]
                    for c in range(NCH):
                        nc.sync.dma_start(p_ch[c][:],
                                          Pd[r][:, c * CW:(c + 1) * CW])

                gpsum = psg.tile([128, BS], f32, tag="g")
                t0c = 0
                for j, csz in enumerate(UCH):
                    # U^T chunk: ut[p, t, b] = ZiT[p, b] * ZT[t, b]
                    zin = zit[:].rearrange("p (o b) -> p o b", o=1
                                           ).broadcast_to((128, csz, BS))
                    ut = upool.tile([128, 18 * BS], bf16, tag=f"u{j % 4}",
                                    name=f"u{j % 4}")
                    ut3 = ut[:, 0:csz * BS].rearrange("p (t b) -> p t b",
                                                      b=BS)
                    nc.vector.tensor_tensor(
                        ut3, zin, ztb3[:, t0c:t0c + csz, :], AL.mult)
                    for i in range(csz):
                        t = t0c + i
                        pc, pi = t // TCH, t % TCH
                        nc.tensor.matmul(
                            gpsum[:],
                            lhsT=p_ch[pc][:, pi * 128:(pi + 1) * 128],
                            rhs=ut3[:, i, :],
                            start=(t == 0), stop=(t == H - 1),
                        )
                    t0c += csz

                # batch stats straight from PSUM via ACT accum_out
                last = (r == RANK - 1)
                stw = 8 if last else 2
                stl = spool.tile([H, stw], f32, tag=f"stl{stw}")
                if last:
                    nc.vector.memset(stl[:], 0.0)
                scr = spool.tile([H, BS], bf16, tag="scr")
                if last:
                    gt = spool.tile([H, BS], bf16, tag="gt")
                    nc.scalar.activation(gt[:], gpsum[:], FT.Copy,
                                         accum_out=stl[:, 0:1])
                else:
                    nc.scalar.activation(scr[:], gpsum[:], FT.Copy,
                                         accum_out=stl[:, 0:1])
                nc.scalar.activation(scr[:], gpsum[:], FT.Square,
                                     accum_out=stl[:, 1:2])
                if last:
                    # piggyback Y-BN inputs on the final sync: with
                    # R = sum_{i<8} Zi (= yt now) and Zi8 = a*G + c,
                    # SumY and SumY^2 expand in closed form from
                    # [S1G, S2G, S1R, S2R, Sum(R*G)] -- no 9th sync.
                    nc.scalar.activation(scr[:], yt[:], FT.Copy,
                                         accum_out=stl[:, 2:3])
                    nc.scalar.activation(scr[:], yt[:], FT.Square,
                                         accum_out=stl[:, 3:4])
                    scry2 = spool.tile([H, BS], bf16, tag="scry2")
                    nc.vector.tensor_tensor(scry2[:], yt[:], gt[:], AL.mult)
                    nc.scalar.activation(scr[:], scry2[:], FT.Copy,
                                         accum_out=stl[:, 4:5])

                # flatten stats into a single partition-0 row [1, stw*128]
                # (the cross-core bounce then needs only 1 DMA descriptor
                # instead of 16 partition-group descriptors = ~6us saved)
                strow = spool.tile([1, 8 * 128], f32, tag="strow",
                                   bufs=1)
                for g in range((stw + 3) // 4):
                    ncol = min(4, stw - 4 * g)
                    pstt = pstr.tile([1, 512], f32, tag="trs", name="pstt")
                    for s4 in range(ncol):
                        s = 4 * g + s4
                        nc.tensor.matmul(pstt[0:1, s4 * 128:(s4 + 1) * 128],
                                         lhsT=stl[:, s:s + 1], rhs=identf[:],
                                         start=True, stop=True)
                    nc.scalar.activation(
                        strow[0:1, g * 512:g * 512 + ncol * 128],
                        pstt[0:1, 0:ncol * 128], FT.Copy)

                # ---- cross-core AllGather of row stats + coeffs ----
                a_ap, c_ap, stg = _bn_sync(nc, tc, dpool, spool, pstr,
                                           strow, stw, bnc, ones11,
                                           epsc=epsc)

                # BN apply fused into the PSUM evacuation:
                # Zi+1^T = a*G^T + c  (per-partition affine on ScalarE)
                zit_next = zitpool.tile([H, BS], bf16, tag="zit")
                nc.scalar.activation(zit_next[:], gpsum[:], FT.Identity,
                                     bias=c_ap, scale=a_ap)
                nc.vector.tensor_tensor(yt[:], yt[:], zit_next[:], AL.add)
                zit = zit_next

            # ------- Y BN from closed-form global sums (no extra sync) ----
            # stg (columns): [S1G, S2G, S1R, S2R, SX]; a_ap/c_ap = rank-7 BN.
            # SumY  = (S1R + a*S1G + B*c) / 8
            # SumY2 = (S2R + 2*(a*SX + c*S1R)
            #          + a^2*S2G + 2*a*c*S1G + B*c^2) / 64
            S1G, S2G = stg[:, 0:1], stg[:, 1:2]
            S1R, S2R = stg[:, 2:3], stg[:, 3:4]
            SX = stg[:, 4:5]
            w = spool.tile([H, 10], f32, tag="ywork")
            nc.vector.tensor_tensor(w[:, 0:1], a_ap, S1G, AL.mult)   # a*S1G
            nc.vector.tensor_scalar(w[:, 1:2], c_ap, float(B), w[:, 0:1],
                                    AL.mult, AL.add)                 # S1Z
            nc.vector.tensor_tensor(w[:, 2:3], w[:, 1:2], S1R, AL.add)  # 8SumY
            nc.vector.tensor_tensor(w[:, 3:4], a_ap, SX, AL.mult)
            nc.vector.tensor_tensor(w[:, 4:5], c_ap, S1R, AL.mult)
            nc.vector.tensor_tensor(w[:, 3:4], w[:, 3:4], w[:, 4:5], AL.add)
            # w3 = SRZ = a*SX + c*S1R
            nc.vector.tensor_tensor(w[:, 5:6], a_ap, a_ap, AL.mult)  # a^2
            nc.vector.tensor_tensor(w[:, 5:6], w[:, 5:6], S2G, AL.mult)
            nc.vector.tensor_tensor(w[:, 6:7], a_ap, c_ap, AL.mult)  # a*c
            nc.vector.tensor_tensor(w[:, 6:7], w[:, 6:7], S1G, AL.mult)
            nc.vector.tensor_tensor(w[:, 7:8], c_ap, c_ap, AL.mult)  # c^2
            nc.vector.tensor_scalar(w[:, 7:8], w[:, 7:8], float(B), None,
                                    AL.mult)
            # S2Z = a^2*S2G + 2*a*c*S1G + B*c^2
            nc.vector.tensor_scalar(w[:, 6:7], w[:, 6:7], 2.0, None, AL.mult)
            nc.vector.tensor_tensor(w[:, 5:6], w[:, 5:6], w[:, 6:7], AL.add)
            nc.vector.tensor_tensor(w[:, 5:6], w[:, 5:6], w[:, 7:8], AL.add)
            nc.vector.tensor_scalar(w[:, 3:4], w[:, 3:4], 2.0, None, AL.mult)
            nc.vector.tensor_tensor(w[:, 8:9], S2R, w[:, 3:4], AL.add)
            nc.vector.tensor_tensor(w[:, 8:9], w[:, 8:9], w[:, 5:6], AL.add)
            # w8 = SumY2*64;  mean/var of Y:
            nc.vector.tensor_scalar(w[:, 2:3], w[:, 2:3], 1.0 / (8.0 * B),
                                    None, AL.mult)                   # mY
            nc.vector.tensor_scalar(w[:, 8:9], w[:, 8:9], 1.0 / (64.0 * B),
                                    None, AL.mult)                   # E[Y^2]
            nc.vector.tensor_tensor(w[:, 9:10], w[:, 2:3], w[:, 2:3], AL.mult)
            nc.vector.tensor_scalar(w[:, 9:10], w[:, 9:10], -1.0, w[:, 8:9],
                                    AL.mult, AL.add)                 # var
            sdy = spool.tile([H, 4], f32, tag="ycoef")
            nc.scalar.activation(sdy[:, 0:1], w[:, 9:10], FT.Sqrt,
                                 bias=epsc[:])
            nc.vector.reciprocal(sdy[:, 1:2], sdy[:, 0:1])
            nc.vector.tensor_tensor(sdy[:, 1:2], sdy[:, 1:2], bnc[:, 2:3],
                                    AL.mult)                         # ay
            nc.vector.tensor_tensor(sdy[:, 2:3], w[:, 2:3], sdy[:, 1:2],
                                    AL.mult)
            nc.vector.tensor_tensor(sdy[:, 2:3], bnc[:, 3:4], sdy[:, 2:3],
                                    AL.subtract)                     # cy
            nc.vector.tensor_scalar(sdy[:, 3:4], sdy[:, 1:2], 0.125, None,
                                    AL.mult)                         # ay/8
            ybn = spool.tile([H, BS], bf16, tag="ybn")
            nc.vector.tensor_scalar(ybn[:], yt[:], sdy[:, 3:4], sdy[:, 2:3],
                                    AL.mult, AL.add)

            # ---------------- final: relu(relu(Y@W3+b3)+relu(X@W2+b2)) ----
            for bt in range(NBT):
                psA = psmm.tile([128, OUT], f32, tag="mm")
                nc.tensor.matmul(psA[:],
                                 lhsT=ybn[:, bt * 128:(bt + 1) * 128],
                                 rhs=w3[:], start=True, stop=True)
                r1 = spool.tile([128, OUT], f32, tag="r1")
                nc.vector.tensor_tensor(r1[:], psA[:], b3b[:], AL.add)
                r1r = spool.tile([128, OUT], f32, tag="r1r")
                nc.scalar.activation(r1r[:], r1[:], FT.Relu)

                s = spool.tile([128, OUT], f32, tag="s")
                nc.vector.tensor_tensor(s[:], r1r[:], r2rs[bt][:], AL.add)
                of = spool.tile([128, OUT], f32, tag="of")
                nc.scalar.activation(of[:], s[:], FT.Relu)
                nc.scalar.dma_start(OUTd[bt * 128:(bt + 1) * 128, :],
                                    of[:])

    nc.compile()
    return nc


def _bn_sync(nc, tc, dpool, spool, pstr, strow, stw, bnc, ones11,
             epsc=None):
    """AllGather per-core [1, stw*128] row stats (single-descriptor DMAs),
    tree-reduce across the 8 cores along the free dim, push the global
    sums back to per-partition columns via contraction-1 matmuls, then
    compute BN coeffs a, c (s.t. BN(x) = a*x + c) in fast column layout.

    Returns (a[128,1], c[128,1], global-sum columns [128, stw])."""
    from concourse import mybir

    f32 = mybir.dt.float32
    FT = mybir.ActivationFunctionType
    AL = mybir.AluOpType

    W = stw * 128
    src = dpool.tile([1, W], f32, tag=f"ccsrc{stw}")
    dst = dpool.tile([N_CORES, W], f32, tag=f"ccdst{stw}")
    nc.scalar.dma_start(src[:], strow[0:1, 0:W])
    nc.gpsimd.collective_compute(
        "AllGather", AL.bypass, replica_groups=[list(range(N_CORES))],
        ins=[src.opt()], outs=[dst.opt()],
    )
    gath = spool.tile([1, N_CORES * 8 * 128], f32, tag="gath", bufs=1)
    nc.scalar.dma_start(
        gath[0:1, 0:N_CORES * W],
        dst[:].rearrange("(o c) w -> o (c w)", o=1))
    # tree-reduce over cores along the free dim, in place
    nc.vector.tensor_tensor(gath[0:1, 0:4 * W], gath[0:1, 0:4 * W],
                            gath[0:1, 4 * W:8 * W], AL.add)
    nc.vector.tensor_tensor(gath[0:1, 0:2 * W], gath[0:1, 0:2 * W],
                            gath[0:1, 2 * W:4 * W], AL.add)
    nc.vector.tensor_tensor(gath[0:1, 0:W], gath[0:1, 0:W],
                            gath[0:1, W:2 * W], AL.add)

    # global sums back to per-partition columns: stw c=1 matmuls
    pstc = pstr.tile([128, 8], f32, tag="trb")
    for s in range(stw):
        nc.tensor.matmul(pstc[:, s:s + 1],
                         lhsT=gath[0:1, s * 128:(s + 1) * 128],
                         rhs=ones11[:], start=True, stop=True)
    st = spool.tile([128, 8], f32, tag="stcol")
    nc.scalar.activation(st[:, 0:stw], pstc[:, 0:stw], FT.Copy)

    cf = spool.tile([H, 8], f32, tag="cf")
    me2 = cf[:, 0:2]   # [mean, E[x^2]]
    m = cf[:, 0:1]
    ex2 = cf[:, 1:2]
    v = cf[:, 2:3]
    sd = cf[:, 3:4]
    rinv = cf[:, 4:5]
    a = cf[:, 5:6]
    t = cf[:, 6:7]
    c = cf[:, 7:8]
    nc.vector.tensor_scalar(me2, st[:, 0:2], 1.0 / B, None, AL.mult)
    msq = spool.tile([H, 1], f32, tag="msq")
    nc.vector.tensor_tensor(msq[:], m, m, AL.mult)
    # v = (msq * -1) + ex2  (one fused tensor_scalar)
    nc.vector.tensor_scalar(v, msq[:], -1.0, ex2, AL.mult, AL.add)
    nc.scalar.activation(sd, v, FT.Sqrt, bias=epsc[:])
    nc.vector.reciprocal(rinv, sd)
    nc.vector.tensor_tensor(a, rinv, bnc[:, 0:1], AL.mult)
    nc.vector.tensor_tensor(t, m, a, AL.mult)
    nc.vector.tensor_tensor(c, bnc[:, 1:2], t, AL.subtract)
    return a, c, st


def _prep_inputs(X, W1, b1, W2, b2, W3, b3, P, gz, bz, gy, by):
    bf = ml_dtypes.bfloat16
    per_core = []
    P_b = np.ascontiguousarray(P.reshape(RANK, H, QK)).astype(bf)
    W1_b = np.ascontiguousarray(W1.reshape(2, 128, H)).astype(bf)
    W2_b = np.ascontiguousarray(W2.reshape(2, 128, OUT)).astype(bf)
    W3_b = np.ascontiguousarray(W3).astype(bf)
    b1b = np.broadcast_to(b1, (128, H)).astype(np.float32).copy()
    b2b = np.broadcast_to(b2, (128, OUT)).astype(np.float32).copy()
    b3b = np.broadcast_to(b3, (128, OUT)).astype(np.float32).copy()
    bnr = np.concatenate([gz, bz, gy, by]).reshape(1, 4 * H
                                                   ).astype(np.float32)
    bnc = np.stack([gz, bz, gy, by], axis=1).astype(np.float32)
    ident = np.eye(128, dtype=np.float32).astype(bf)
    identf = np.eye(128, dtype=np.float32)
    for s in range(N_CORES):
        Xs = X[s * BS:(s + 1) * BS]
        XT = np.ascontiguousarray(Xs.T.reshape(2, 128, BS)).astype(bf)
        per_core.append({
            "XT": XT, "P": P_b, "W1": W1_b, "W2": W2_b, "W3": W3_b,
            "b1b": b1b, "b2b": b2b, "b3b": b3b, "bnr": bnr, "bnc": bnc,
            "ident": ident, "identf": identf,
        })
    return per_core


def kernel(**inputs):
    _ensure_axon_hooks_shim()
    from concourse.bass_utils import run_bass_kernel_spmd

    if "nc" not in _cache:
        _cache["nc"] = _build()
    nc = _cache["nc"]

    in_maps = _prep_inputs(**{k: np.asarray(v) for k, v in inputs.items()})
    res = run_bass_kernel_spmd(nc, in_maps, core_ids=list(range(N_CORES)))
    out = np.concatenate([m["out"] for m in res.results], axis=0)
    return out.astype(np.float32)


if __name__ == "__main__":
    import reference as R

    inputs = {k: np.asarray(v) for k, v in R.setup_inputs().items()}
    got = kernel(**inputs)
    exp = np.asarray(R.reference(**R.setup_inputs()))
    rel = np.linalg.norm(got - exp) / np.linalg.norm(exp)
    print("rel l2:", rel)
